# revision 3
# baseline (speedup 1.0000x reference)
"""Integrate-and-fire scan (T=8) on Trainium2, data-parallel over 8 NeuronCores.

Reference semantics per element, scanned over t:
    mem = mem + x[t]; spike = (mem - 1 > 0); mem = mem - spike

Sharding: batch dim (axis 1 of x / axis 0 of mem0) split 4-per-core across 8
cores; the scan is elementwise so no cross-core communication is needed.

Per core the shard is viewed as [T=8, P=128, F=4704] (4*3*224*224 = 602112 =
128*4704). The kernel streams column chunks: membrane chunk stays resident in
SBUF while the 8 timesteps are applied (TT add, TS is_gt, TT sub on VectorE),
spikes DMA out per timestep. DMA-bound: ~41 MB HBM traffic per core.
"""

import sys

if "/opt/trn_rl_repo" not in sys.path:
    sys.path.insert(0, "/opt/trn_rl_repo")

import numpy as np

import concourse.bass as bass  # noqa: F401  (registers engine classes)
import concourse.tile as tile
from concourse import bacc, mybir
from concourse.bass_utils import run_bass_kernel_spmd

T, B, C, H, W = 8, 32, 3, 224, 224
NCORES = 8
BPC = B // NCORES            # 4 batch elements per core
E = BPC * C * H * W          # 602112 elements per (core, timestep)
P = 128
F = E // P                   # 4704 free-dim columns
F32 = mybir.dt.float32

# Tunables
CHUNK_W = 588                # columns per chunk (divides 4704)
N_CHUNKS = F // CHUNK_W
X_BUFS = 10                  # x-tile double buffering depth
S_BUFS = 10                  # spike-tile buffering depth
SPIKE_ENGINE = "gpsimd"      # which engine computes is_gt: "vector"|"gpsimd"

_compiled_nc = None


def _build():
    nc = bacc.Bacc("TRN2", target_bir_lowering=False, debug=False,
                   num_devices=NCORES)
    x = nc.dram_tensor("x", [T, P, F], F32, kind="ExternalInput").ap()
    m0 = nc.dram_tensor("mem0", [P, F], F32, kind="ExternalInput").ap()
    out = nc.dram_tensor("out", [T, P, F], F32, kind="ExternalOutput").ap()

    with tile.TileContext(nc) as tc:
        with tc.tile_pool(name="mem", bufs=N_CHUNKS) as mem_pool, \
             tc.tile_pool(name="xin", bufs=X_BUFS) as x_pool, \
             tc.tile_pool(name="spk", bufs=S_BUFS) as s_pool:
            mts = []
            for c in range(N_CHUNKS):
                mt = mem_pool.tile([P, CHUNK_W], F32)
                nc.sync.dma_start(out=mt[:], in_=m0[:, bass.ts(c, CHUNK_W)])
                mts.append(mt)
            for c in range(N_CHUNKS):
                sl = bass.ts(c, CHUNK_W)
                mt = mts[c]
                for t in range(T):
                    xt = x_pool.tile([P, CHUNK_W], F32)
                    nc.sync.dma_start(out=xt[:], in_=x[t, :, sl])
                    nc.vector.tensor_add(mt[:], mt[:], xt[:])
                    st = s_pool.tile([P, CHUNK_W], F32)
                    if SPIKE_ENGINE == "gpsimd":
                        nc.gpsimd.tensor_scalar(
                            out=st[:], in0=mt[:], scalar1=1.0, scalar2=None,
                            op0=mybir.AluOpType.is_gt)
                    else:
                        nc.vector.tensor_scalar(
                            out=st[:], in0=mt[:], scalar1=1.0, scalar2=None,
                            op0=mybir.AluOpType.is_gt)
                    nc.vector.tensor_sub(mt[:], mt[:], st[:])
                    nc.scalar.dma_start(out=out[t, :, sl], in_=st[:])
    nc.compile()
    return nc


def _get_nc():
    global _compiled_nc
    if _compiled_nc is None:
        _compiled_nc = _build()
    return _compiled_nc


def _run(x, mem0, trace=False):
    nc = _get_nc()
    in_maps = []
    for i in range(NCORES):
        bsl = slice(i * BPC, (i + 1) * BPC)
        xi = np.ascontiguousarray(x[:, bsl]).reshape(T, P, F)
        mi = np.ascontiguousarray(mem0[bsl]).reshape(P, F)
        in_maps.append({"x": xi, "mem0": mi})
    res = run_bass_kernel_spmd(nc, in_maps, list(range(NCORES)), trace=trace)
    shards = [res.results[i]["out"].reshape(T, BPC, C, H, W)
              for i in range(NCORES)]
    full = np.concatenate(shards, axis=1)
    return full, res


def kernel(x, mem0):
    x = np.asarray(x, dtype=np.float32)
    mem0 = np.asarray(mem0, dtype=np.float32)
    full, _ = _run(x, mem0, trace=False)
    return full


# revision 6
# speedup vs baseline: 3.6108x; 3.6108x over previous
"""Integrate-and-fire scan (T=8) on Trainium2, data-parallel over 8 NeuronCores.

Reference semantics per element, scanned over t:
    mem = mem + x[t]; spike = (mem - 1 > 0); mem = mem - spike

Sharding: batch dim (axis 1 of x / axis 0 of mem0) split 4-per-core across 8
cores; the scan is elementwise so no cross-core communication is needed.

Per core the shard is viewed as [T=8, P=128, F=4704] (4*3*224*224 = 602112 =
128*4704). The kernel streams column chunks: membrane chunk stays resident in
SBUF while the 8 timesteps are applied (TT add, TS is_gt, TT sub on VectorE),
spikes DMA out per timestep. DMA-bound: ~41 MB HBM traffic per core.
"""

import sys

if "/opt/trn_rl_repo" not in sys.path:
    sys.path.insert(0, "/opt/trn_rl_repo")

import numpy as np

import concourse.bass as bass  # noqa: F401  (registers engine classes)
import concourse.tile as tile
from concourse import bacc, mybir
from concourse.bass_utils import run_bass_kernel_spmd

T, B, C, H, W = 8, 32, 3, 224, 224
NCORES = 8
BPC = B // NCORES            # 4 batch elements per core
E = BPC * C * H * W          # 602112 elements per (core, timestep)
P = 128
F = E // P                   # 4704 free-dim columns
F32 = mybir.dt.float32

# Tunables
CHUNK_W = 1176               # columns per chunk (divides 4704)
N_CHUNKS = F // CHUNK_W
X_BUFS = 8                   # x-tile double buffering depth
S_BUFS = 6                   # spike-tile buffering depth
SPIKE_ENGINE = "scalar"      # "vector" (is_gt on DVE) | "scalar" (Sign+Relu on ACT)
OUT_DMA_ENGINE = "gpsimd"    # "scalar" | "sync" | "gpsimd"

_compiled_nc = None


def _build():
    nc = bacc.Bacc("TRN2", target_bir_lowering=False, debug=False,
                   num_devices=NCORES)
    x = nc.dram_tensor("x", [T, P, F], F32, kind="ExternalInput").ap()
    m0 = nc.dram_tensor("mem0", [P, F], F32, kind="ExternalInput").ap()
    out = nc.dram_tensor("out", [T, P, F], F32, kind="ExternalOutput").ap()

    if SPIKE_ENGINE == "scalar":
        # activation() lowers float bias to a const AP; -1.0 isn't in the
        # default database, so register it (same pattern as Bass.__init__).
        neg1 = nc.alloc_sbuf_tensor("const-float32--1.0", [128, 1], F32)
        nc.gpsimd.memset(neg1.ap(), -1.0)
        nc.const_aps.aps[(F32, -1.0)] = neg1.ap()
        nc.all_engine_barrier()

    with tile.TileContext(nc) as tc:
        with tc.tile_pool(name="mem", bufs=N_CHUNKS) as mem_pool, \
             tc.tile_pool(name="xin", bufs=X_BUFS) as x_pool, \
             tc.tile_pool(name="spk", bufs=S_BUFS) as s_pool:
            mts = []
            for c in range(N_CHUNKS):
                mt = mem_pool.tile([P, CHUNK_W], F32)
                nc.sync.dma_start(out=mt[:], in_=m0[:, bass.ts(c, CHUNK_W)])
                mts.append(mt)
            out_dma = {"scalar": nc.scalar, "sync": nc.sync,
                       "gpsimd": nc.gpsimd}[OUT_DMA_ENGINE]
            for c in range(N_CHUNKS):
                sl = bass.ts(c, CHUNK_W)
                mt = mts[c]
                for t in range(T):
                    xt = x_pool.tile([P, CHUNK_W], F32)
                    nc.sync.dma_start(out=xt[:], in_=x[t, :, sl])
                    nc.vector.tensor_add(mt[:], mt[:], xt[:])
                    st = s_pool.tile([P, CHUNK_W], F32)
                    if SPIKE_ENGINE == "scalar":
                        # spike = relu(sign(mem' - 1)) in {0,1}, exactly
                        # (u > 1); both steps on the otherwise-idle ScalarE.
                        nc.scalar.activation(
                            st[:], mt[:], mybir.ActivationFunctionType.Sign,
                            bias=-1.0, scale=1.0)
                        nc.scalar.activation(
                            st[:], st[:], mybir.ActivationFunctionType.Relu)
                    else:
                        nc.vector.tensor_scalar(
                            out=st[:], in0=mt[:], scalar1=1.0, scalar2=None,
                            op0=mybir.AluOpType.is_gt)
                    nc.vector.tensor_sub(mt[:], mt[:], st[:])
                    out_dma.dma_start(out=out[t, :, sl], in_=st[:])
    nc.compile()
    return nc


def _get_nc():
    global _compiled_nc
    if _compiled_nc is None:
        _compiled_nc = _build()
    return _compiled_nc


def _run(x, mem0, trace=False):
    nc = _get_nc()
    in_maps = []
    for i in range(NCORES):
        bsl = slice(i * BPC, (i + 1) * BPC)
        xi = np.ascontiguousarray(x[:, bsl]).reshape(T, P, F)
        mi = np.ascontiguousarray(mem0[bsl]).reshape(P, F)
        in_maps.append({"x": xi, "mem0": mi})
    res = run_bass_kernel_spmd(nc, in_maps, list(range(NCORES)), trace=trace)
    shards = [res.results[i]["out"].reshape(T, BPC, C, H, W)
              for i in range(NCORES)]
    full = np.concatenate(shards, axis=1)
    return full, res


def kernel(x, mem0):
    x = np.asarray(x, dtype=np.float32)
    mem0 = np.asarray(mem0, dtype=np.float32)
    full, _ = _run(x, mem0, trace=False)
    return full


# revision 8
# speedup vs baseline: 5.2734x; 1.4605x over previous
"""Integrate-and-fire scan (T=8) on Trainium2, data-parallel over 8 NeuronCores.

Reference semantics per element, scanned over t:
    mem = mem + x[t]; spike = (mem - 1 > 0); mem = mem - spike

Sharding: batch dim (axis 1 of x / axis 0 of mem0) split 4-per-core across 8
cores; the scan is elementwise so no cross-core communication is needed.

Per core the shard is viewed as [T=8, P=128, F=4704] (4*3*224*224 = 602112 =
128*4704). The kernel streams column chunks: membrane chunk stays resident in
SBUF while the 8 timesteps are applied (TT add, TS is_gt, TT sub on VectorE),
spikes DMA out per timestep. DMA-bound: ~41 MB HBM traffic per core.
"""

import sys

if "/opt/trn_rl_repo" not in sys.path:
    sys.path.insert(0, "/opt/trn_rl_repo")

import numpy as np

import concourse.bass as bass  # noqa: F401  (registers engine classes)
import concourse.tile as tile
from concourse import bacc, mybir
from concourse.bass_utils import run_bass_kernel_spmd

T, B, C, H, W = 8, 32, 3, 224, 224
NCORES = 8
BPC = B // NCORES            # 4 batch elements per core
E = BPC * C * H * W          # 602112 elements per (core, timestep)
P = 128
F = E // P                   # 4704 free-dim columns
F32 = mybir.dt.float32

# Tunables
CHUNK_W = 588                # columns per chunk (divides 4704)
N_CHUNKS = F // CHUNK_W
X_BUFS = 2 * N_CHUNKS        # x-tile buffering depth (two t-rounds)
S_BUFS = 2 * N_CHUNKS        # spike-tile buffering depth
SPIKE_ENGINE = "scalar"      # "vector" (is_gt on DVE) | "scalar" (Sign+Relu on ACT)
OUT_DMA_ENGINE = "gpsimd"    # "scalar" | "sync" | "gpsimd"

_compiled_nc = None


def _build():
    nc = bacc.Bacc("TRN2", target_bir_lowering=False, debug=False,
                   num_devices=NCORES)
    x = nc.dram_tensor("x", [T, P, F], F32, kind="ExternalInput").ap()
    m0 = nc.dram_tensor("mem0", [P, F], F32, kind="ExternalInput").ap()
    out = nc.dram_tensor("out", [T, P, F], F32, kind="ExternalOutput").ap()

    if SPIKE_ENGINE == "scalar":
        # activation() lowers float bias to a const AP; -1.0 isn't in the
        # default database, so register it (same pattern as Bass.__init__).
        neg1 = nc.alloc_sbuf_tensor("const-float32--1.0", [128, 1], F32)
        nc.gpsimd.memset(neg1.ap(), -1.0)
        nc.const_aps.aps[(F32, -1.0)] = neg1.ap()
        nc.all_engine_barrier()

    with tile.TileContext(nc) as tc:
        with tc.tile_pool(name="mem", bufs=N_CHUNKS) as mem_pool, \
             tc.tile_pool(name="xin", bufs=X_BUFS) as x_pool, \
             tc.tile_pool(name="spk", bufs=S_BUFS) as s_pool:
            mts = []
            for c in range(N_CHUNKS):
                mt = mem_pool.tile([P, CHUNK_W], F32)
                nc.sync.dma_start(out=mt[:], in_=m0[:, bass.ts(c, CHUNK_W)])
                mts.append(mt)
            out_dma = {"scalar": nc.scalar, "sync": nc.sync,
                       "gpsimd": nc.gpsimd}[OUT_DMA_ENGINE]
            # t-outer, chunk-inner, phase-grouped issue order: engines run
            # their instruction streams in order, so grouping each phase
            # across chunks keeps every engine stall-free (chunk c's spike
            # computes while chunk c+1 adds, etc).
            for t in range(T):
                xts, sts = [], []
                for c in range(N_CHUNKS):
                    xt = x_pool.tile([P, CHUNK_W], F32)
                    nc.sync.dma_start(out=xt[:], in_=x[t, :, bass.ts(c, CHUNK_W)])
                    xts.append(xt)
                for c in range(N_CHUNKS):
                    nc.vector.tensor_add(mts[c][:], mts[c][:], xts[c][:])
                for c in range(N_CHUNKS):
                    st = s_pool.tile([P, CHUNK_W], F32)
                    sts.append(st)
                    if SPIKE_ENGINE == "scalar":
                        # spike = relu(sign(mem' - 1)) in {0,1}, exactly
                        # (u > 1); both steps on the otherwise-idle ScalarE.
                        nc.scalar.activation(
                            st[:], mts[c][:], mybir.ActivationFunctionType.Sign,
                            bias=-1.0, scale=1.0)
                        nc.scalar.activation(
                            st[:], st[:], mybir.ActivationFunctionType.Relu)
                    else:
                        nc.vector.tensor_scalar(
                            out=st[:], in0=mts[c][:], scalar1=1.0, scalar2=None,
                            op0=mybir.AluOpType.is_gt)
                for c in range(N_CHUNKS):
                    nc.vector.tensor_sub(mts[c][:], mts[c][:], sts[c][:])
                    out_dma.dma_start(out=out[t, :, bass.ts(c, CHUNK_W)],
                                      in_=sts[c][:])
    nc.compile()
    return nc


def _get_nc():
    global _compiled_nc
    if _compiled_nc is None:
        _compiled_nc = _build()
    return _compiled_nc


def _run(x, mem0, trace=False):
    nc = _get_nc()
    in_maps = []
    for i in range(NCORES):
        bsl = slice(i * BPC, (i + 1) * BPC)
        xi = np.ascontiguousarray(x[:, bsl]).reshape(T, P, F)
        mi = np.ascontiguousarray(mem0[bsl]).reshape(P, F)
        in_maps.append({"x": xi, "mem0": mi})
    res = run_bass_kernel_spmd(nc, in_maps, list(range(NCORES)), trace=trace)
    shards = [res.results[i]["out"].reshape(T, BPC, C, H, W)
              for i in range(NCORES)]
    full = np.concatenate(shards, axis=1)
    return full, res


def kernel(x, mem0):
    x = np.asarray(x, dtype=np.float32)
    mem0 = np.asarray(mem0, dtype=np.float32)
    full, _ = _run(x, mem0, trace=False)
    return full


# revision 11
# speedup vs baseline: 5.2949x; 1.0041x over previous
"""Integrate-and-fire scan (T=8) on Trainium2, data-parallel over 8 NeuronCores.

Reference semantics per element, scanned over t:
    mem = mem + x[t]; spike = (mem - 1 > 0); mem = mem - spike

Sharding: batch dim (axis 1 of x / axis 0 of mem0) split 4-per-core across 8
cores; the scan is elementwise so no cross-core communication is needed.

Per core the shard is viewed as [T=8, P=128, F=4704] (4*3*224*224 = 602112 =
128*4704). The kernel streams column chunks: membrane chunk stays resident in
SBUF while the 8 timesteps are applied (TT add, TS is_gt, TT sub on VectorE),
spikes DMA out per timestep. DMA-bound: ~41 MB HBM traffic per core.
"""

import sys

if "/opt/trn_rl_repo" not in sys.path:
    sys.path.insert(0, "/opt/trn_rl_repo")

import numpy as np

import concourse.bass as bass  # noqa: F401  (registers engine classes)
import concourse.tile as tile
from concourse import bacc, mybir
from concourse.bass_utils import run_bass_kernel_spmd

T, B, C, H, W = 8, 32, 3, 224, 224
NCORES = 8
BPC = B // NCORES            # 4 batch elements per core
E = BPC * C * H * W          # 602112 elements per (core, timestep)
P = 128
F = E // P                   # 4704 free-dim columns
F32 = mybir.dt.float32

import os

# Tunables (env-overridable for A/B testing)
CHUNK_W = int(os.environ.get("IAF_CHUNK_W", "588"))   # columns/chunk (divides 4704)
N_CHUNKS = F // CHUNK_W
X_BUFS = int(os.environ.get("IAF_X_BUFS", str(2 * N_CHUNKS)))
S_BUFS = int(os.environ.get("IAF_S_BUFS", str(2 * N_CHUNKS)))
SPIKE_ENGINE = os.environ.get("IAF_SPIKE", "scalar")  # "vector" | "scalar"
OUT_DMA_ENGINE = os.environ.get("IAF_OUT_DMA", "gpsimd")   # scalar|sync|gpsimd
MEM0_DMA_ENGINE = os.environ.get("IAF_MEM0_DMA", "sync")   # scalar|sync|gpsimd

_compiled_nc = None


def _build():
    nc = bacc.Bacc("TRN2", target_bir_lowering=False, debug=False,
                   num_devices=NCORES)
    x = nc.dram_tensor("x", [T, P, F], F32, kind="ExternalInput").ap()
    m0 = nc.dram_tensor("mem0", [P, F], F32, kind="ExternalInput").ap()
    out = nc.dram_tensor("out", [T, P, F], F32, kind="ExternalOutput").ap()

    if SPIKE_ENGINE == "scalar":
        # activation() lowers float bias to a const AP; -1.0 isn't in the
        # default database, so register it (same pattern as Bass.__init__).
        neg1 = nc.alloc_sbuf_tensor("const-float32--1.0", [128, 1], F32)
        nc.gpsimd.memset(neg1.ap(), -1.0)
        nc.const_aps.aps[(F32, -1.0)] = neg1.ap()
        nc.all_engine_barrier()

    with tile.TileContext(nc) as tc:
        with tc.tile_pool(name="mem", bufs=N_CHUNKS) as mem_pool, \
             tc.tile_pool(name="xin", bufs=X_BUFS) as x_pool, \
             tc.tile_pool(name="spk", bufs=S_BUFS) as s_pool:
            eng = {"scalar": nc.scalar, "sync": nc.sync, "gpsimd": nc.gpsimd}
            mem0_dma = eng[MEM0_DMA_ENGINE]
            mts = []
            for c in range(N_CHUNKS):
                mt = mem_pool.tile([P, CHUNK_W], F32)
                mem0_dma.dma_start(out=mt[:], in_=m0[:, bass.ts(c, CHUNK_W)])
                mts.append(mt)
            out_dma = eng[OUT_DMA_ENGINE]
            # t-outer, chunk-inner, phase-grouped issue order: engines run
            # their instruction streams in order, so grouping each phase
            # across chunks keeps every engine stall-free (chunk c's spike
            # computes while chunk c+1 adds, etc).
            for t in range(T):
                xts, sts = [], []
                for c in range(N_CHUNKS):
                    xt = x_pool.tile([P, CHUNK_W], F32)
                    nc.sync.dma_start(out=xt[:], in_=x[t, :, bass.ts(c, CHUNK_W)])
                    xts.append(xt)
                for c in range(N_CHUNKS):
                    nc.vector.tensor_add(mts[c][:], mts[c][:], xts[c][:])
                for c in range(N_CHUNKS):
                    st = s_pool.tile([P, CHUNK_W], F32)
                    sts.append(st)
                    if SPIKE_ENGINE == "scalar":
                        # spike = relu(sign(mem' - 1)) in {0,1}, exactly
                        # (u > 1); both steps on the otherwise-idle ScalarE.
                        nc.scalar.activation(
                            st[:], mts[c][:], mybir.ActivationFunctionType.Sign,
                            bias=-1.0, scale=1.0)
                        nc.scalar.activation(
                            st[:], st[:], mybir.ActivationFunctionType.Relu)
                    else:
                        nc.vector.tensor_scalar(
                            out=st[:], in0=mts[c][:], scalar1=1.0, scalar2=None,
                            op0=mybir.AluOpType.is_gt)
                for c in range(N_CHUNKS):
                    nc.vector.tensor_sub(mts[c][:], mts[c][:], sts[c][:])
                    out_dma.dma_start(out=out[t, :, bass.ts(c, CHUNK_W)],
                                      in_=sts[c][:])
    nc.compile()
    return nc


def _get_nc():
    global _compiled_nc
    if _compiled_nc is None:
        _compiled_nc = _build()
    return _compiled_nc


def _run(x, mem0, trace=False):
    nc = _get_nc()
    in_maps = []
    for i in range(NCORES):
        bsl = slice(i * BPC, (i + 1) * BPC)
        xi = np.ascontiguousarray(x[:, bsl]).reshape(T, P, F)
        mi = np.ascontiguousarray(mem0[bsl]).reshape(P, F)
        in_maps.append({"x": xi, "mem0": mi})
    res = run_bass_kernel_spmd(nc, in_maps, list(range(NCORES)), trace=trace)
    shards = [res.results[i]["out"].reshape(T, BPC, C, H, W)
              for i in range(NCORES)]
    full = np.concatenate(shards, axis=1)
    return full, res


def kernel(x, mem0):
    x = np.asarray(x, dtype=np.float32)
    mem0 = np.asarray(mem0, dtype=np.float32)
    full, _ = _run(x, mem0, trace=False)
    return full
